# revision 13
# baseline (speedup 1.0000x reference)
"""2-layer LSTM (H=100, D=13, T=131072) + linear head on 8 TRN2 cores.

Strategy: warm-up based sequence parallelism. The sequence is split into
4096 chunks of L=32 steps; each chunk is computed independently starting
from zero state W=32 steps early (forget-gate contraction makes the
trajectory converge to the exact one well within W steps, validated at
fp32 noise floor). No cross-core communication. Each core processes 512
chunks as 2 interleaved streams of 256-wide batched recurrences
(hidden dim on partitions, chunk batch on the free axis).

All activations are sigmoids: tanh(z) = 2*sig(2z)-1 is folded into
host-side weight scaling plus one fused scalar_tensor_tensor per use.
Cell/hidden state are kept halved (c~=c/2, h~=h/2) so the fusions are
exact powers of two.
"""

import os
import sys
from contextlib import ExitStack

import numpy as np

if "/opt/trn_rl_repo" not in sys.path:
    sys.path.insert(0, "/opt/trn_rl_repo")

import concourse.bass as bass
import concourse.bacc as bacc
import concourse.tile as tile
from concourse import mybir
from concourse.bass_utils import run_bass_kernel_spmd

T, D, H = 131072, 13, 100
NCORES = 8
L, WU = 32, 32
TEXT = L + WU                      # rounds per layer
S = 2                              # streams per core
B = 256                            # chunk batch per stream
assert NCORES * S * B * L == T

F32 = mybir.dt.float32
MM_DT = mybir.dt.float32r          # 1 cyc/row on PE for N>=256
AF = mybir.ActivationFunctionType
OP = mybir.AluOpType

LAST_RESULTS = None
_NC_CACHE = {}


def _to_f32r(a):
    """Round fp32 to the fp32r grid (11-bit mantissa, RNE) host-side so the
    DMA'd bits already satisfy the FP32r matmul producer rule."""
    u = np.ascontiguousarray(a, np.float32).view(np.uint32).copy()
    low = u & np.uint32(0xFFF)
    base = u & np.uint32(0xFFFFF000)
    up = (low > 0x800) | ((low == 0x800) & (((base >> np.uint32(12)) & 1) == 1))
    base = base + np.where(up, np.uint32(0x1000), np.uint32(0))
    return base.view(np.float32)


def _build_nc():
    nc = bacc.Bacc("TRN2", target_bir_lowering=False, debug=False)
    xh_d = nc.dram_tensor("xh", [S, 14, TEXT * B], MM_DT, kind="ExternalInput").ap()
    g1_d = nc.dram_tensor("g1", [114, 400], MM_DT, kind="ExternalInput").ap()
    g2a_d = nc.dram_tensor("g2a", [101, 400], MM_DT, kind="ExternalInput").ap()
    g2b_d = nc.dram_tensor("g2b", [100, 400], MM_DT, kind="ExternalInput").ap()
    gfc_d = nc.dram_tensor("gfc", [101, 1], MM_DT, kind="ExternalInput").ap()
    y_d = nc.dram_tensor("y", [S, TEXT, B], F32, kind="ExternalOutput").ap()

    with tile.TileContext(nc) as tc, ExitStack() as ctx:
        pers = ctx.enter_context(tc.tile_pool(name="pers", bufs=1))
        # moving tile stream: rows 0:100 h~ (block t+1 written at round t),
        # rows 100:113 x_t, row 113 ones — block t is round t's matmul rhs
        h1s = [
            pers.tile([114, (TEXT + 1) * B], MM_DT, name=f"h1s{s}") for s in range(S)
        ]
        ct1 = [pers.tile([100, B], F32, name=f"ct1_{s}") for s in range(S)]
        ct2 = [pers.tile([100, B], F32, name=f"ct2_{s}") for s in range(S)]
        ring = [pers.tile([101, 4 * B], MM_DT, name=f"ring{s}") for s in range(S)]
        G1 = pers.tile([114, 400], MM_DT)
        G2a = pers.tile([101, 400], MM_DT)
        G2b = pers.tile([100, 400], MM_DT)
        Gfc = pers.tile([101, 1], MM_DT)

        psum = [
            ctx.enter_context(tc.tile_pool(name=f"ps{s}", bufs=2, space="PSUM"))
            for s in range(S)
        ]
        sigp = [
            ctx.enter_context(tc.tile_pool(name=f"sig{s}", bufs=2)) for s in range(S)
        ]
        tmpp = [
            ctx.enter_context(tc.tile_pool(name=f"tmp{s}", bufs=2)) for s in range(S)
        ]

        nc.sync.dma_start(G1[:], g1_d[:])
        nc.sync.dma_start(G2a[:], g2a_d[:])
        nc.sync.dma_start(G2b[:], g2b_d[:])
        nc.sync.dma_start(Gfc[:], gfc_d[:])
        # Memset cannot emit float32r, so f32r tiles are initialized by
        # copying from an fp32 scratch (copy converts on write). The ring
        # ones row lives at partition 100 (not a legal compute start
        # partition): fill all 101 rows with 1.0 first, then zero the h
        # rows of block 0; blocks 1-3 h rows are written before read.
        sc0 = pers.tile([128, 4 * B], F32, name="sc0")
        nc.vector.memset(sc0[:], 1.0)
        for s in range(S):
            nc.vector.tensor_copy(ring[s][:], sc0[0:101, :])
        nc.vector.memset(sc0[:, 0:B], 0.0)
        for s in range(S):
            nc.sync.dma_start(h1s[s][100:114, 0 : TEXT * B], xh_d[s])
            nc.vector.tensor_copy(h1s[s][0:100, 0:B], sc0[0:100, 0:B])
            nc.vector.memset(ct1[s][:], 0.0)
            nc.vector.memset(ct2[s][:], 0.0)
            nc.vector.tensor_copy(ring[s][0:100, 0:B], sc0[0:100, 0:B])

        def cell_tail(s, sig, ct, h_out):
            """it = (sig_g - 0.5)*sig_i ; ct' = sig_f*ct + it ;
            h~ = (sig(4ct') - 0.5) * sig_o  -> h_out"""
            it2 = tmpp[s].tile([100, B], F32)
            nc.vector.scalar_tensor_tensor(
                it2[:], sig[:, 2 * B : 3 * B], 0.5, sig[:, 0:B],
                op0=OP.subtract, op1=OP.mult,
            )
            fc = tmpp[s].tile([100, B], F32)
            nc.vector.tensor_tensor(fc[:], sig[:, B : 2 * B], ct[:], op=OP.mult)
            nc.vector.tensor_tensor(ct[:], fc[:], it2[:], op=OP.add)
            sc = tmpp[s].tile([100, B], F32)
            nc.scalar.activation(sc[:], ct[:], AF.Sigmoid, scale=4.0)
            nc.vector.scalar_tensor_tensor(
                h_out, sc[:], 0.5, sig[:, 3 * B : 4 * B],
                op0=OP.subtract, op1=OP.mult,
            )

        # ---- layer 1 ----
        for t in range(TEXT):
            for s in range(S):
                mov = h1s[s][:, t * B : (t + 1) * B]
                ps = psum[s].tile([100, 4 * B], F32)
                for g in range(4):
                    nc.tensor.matmul(
                        ps[:, g * B : (g + 1) * B],
                        G1[:, g * 100 : (g + 1) * 100],
                        mov,
                        start=True, stop=True,
                    )
                sig = sigp[s].tile([100, 4 * B], F32)
                nc.scalar.activation(sig[:], ps[:], AF.Sigmoid)
                cell_tail(s, sig, ct1[s], h1s[s][0:100, (t + 1) * B : (t + 2) * B])

        # ---- layer 2 (+ fused output head) ----
        for t in range(TEXT):
            for s in range(S):
                h1blk = h1s[s][0:100, (t + 1) * B : (t + 2) * B]
                r0 = (t % 4) * B
                r1 = ((t + 1) % 4) * B
                mov = ring[s][:, r0 : r0 + B]
                ps = psum[s].tile([100, 4 * B], F32)
                for g in range(4):
                    sl = slice(g * B, (g + 1) * B)
                    nc.tensor.matmul(
                        ps[:, sl], G2b[:, g * 100 : (g + 1) * 100], h1blk,
                        start=True, stop=False,
                    )
                    nc.tensor.matmul(
                        ps[:, sl], G2a[:, g * 100 : (g + 1) * 100], mov,
                        start=False, stop=True,
                    )
                sig = sigp[s].tile([100, 4 * B], F32)
                nc.scalar.activation(sig[:], ps[:], AF.Sigmoid)
                cell_tail(s, sig, ct2[s], ring[s][0:100, r1 : r1 + B])
                # y_t = Gfc.T @ [h~2_t; 1]; reuse a free corner of the psum tile
                nc.tensor.matmul(
                    ps[0:1, 0:B], Gfc[:], ring[s][:, r1 : r1 + B],
                    start=True, stop=True,
                )
                yt = tmpp[s].tile([1, B], F32, name=f"yt{s}")
                nc.vector.tensor_copy(yt[:], ps[0:1, 0:B])
                nc.sync.dma_start(y_d[s, t], yt[:])

    nc.compile()
    return nc


def get_nc():
    if "nc" not in _NC_CACHE:
        _NC_CACHE["nc"] = _build_nc()
    return _NC_CACHE["nc"]


def _prep_weights(inputs):
    f = np.float32
    G1 = np.zeros((114, 400), f)
    G1[0:100] = 2.0 * inputs["W_hh1"].T
    G1[100:113] = inputs["W_ih1"].T
    G1[113] = inputs["b_ih1"] + inputs["b_hh1"]
    G1[:, 200:300] *= 2.0
    G2a = np.zeros((101, 400), f)
    G2a[0:100] = 2.0 * inputs["W_hh2"].T
    G2a[100] = inputs["b_ih2"] + inputs["b_hh2"]
    G2a[:, 200:300] *= 2.0
    G2b = np.ascontiguousarray(2.0 * inputs["W_ih2"].T).astype(f)
    G2b[:, 200:300] *= 2.0
    Gfc = np.zeros((101, 1), f)
    Gfc[0:100, 0] = 2.0 * inputs["W_fc"][0]
    Gfc[100, 0] = inputs["b_fc"][0]
    return G1, G2a, G2b, Gfc


def _prep_x(x):
    f = np.float32
    c_all = np.arange(NCORES * S * B).reshape(NCORES, S, B)
    gidx = c_all[..., None] * L - WU + np.arange(TEXT)          # [K,S,B,TEXT]
    valid = gidx >= 0
    xv = np.where(
        valid[..., None], x[np.clip(gidx, 0, T - 1)], np.float32(0.0)
    )                                                           # [K,S,B,TEXT,13]
    xh = np.empty((NCORES, S, 14, TEXT, B), f)
    xh[:, :, 0:13] = xv.transpose(0, 1, 4, 3, 2)
    xh[:, :, 13] = 1.0
    return xh.reshape(NCORES, S, 14, TEXT * B)


def _host_prefix(inputs, n):
    """Exact first-n outputs in plain numpy (chunk 0 has no warm-up window)."""
    f = np.float32
    x = inputs["x"].astype(f)

    def sig(v):
        return 1.0 / (1.0 + np.exp(-v))

    def layer(xs, Wih, Whh, b):
        h = np.zeros(H, f)
        c = np.zeros(H, f)
        hs = np.empty((n, H), f)
        for t in range(n):
            g = (Wih @ xs[t] + b + Whh @ h).astype(f)
            i, fg, gg, o = g[:H], g[H : 2 * H], g[2 * H : 3 * H], g[3 * H :]
            c = (sig(fg) * c + sig(i) * np.tanh(gg)).astype(f)
            h = (sig(o) * np.tanh(c)).astype(f)
            hs[t] = h
        return hs

    h1 = layer(x[:n], inputs["W_ih1"], inputs["W_hh1"], inputs["b_ih1"] + inputs["b_hh1"])
    h2 = layer(h1, inputs["W_ih2"], inputs["W_hh2"], inputs["b_ih2"] + inputs["b_hh2"])
    return (h2 @ inputs["W_fc"].T + inputs["b_fc"]).astype(f)


def kernel(**inputs) -> np.ndarray:
    global LAST_RESULTS
    inputs = {k: np.asarray(v, dtype=np.float32) for k, v in inputs.items()}
    G1, G2a, G2b, Gfc = (_to_f32r(g) for g in _prep_weights(inputs))
    xh = _to_f32r(_prep_x(inputs["x"]))
    in_maps = [
        {"xh": xh[k], "g1": G1, "g2a": G2a, "g2b": G2b, "gfc": Gfc}
        for k in range(NCORES)
    ]
    nc = get_nc()
    trace = bool(os.environ.get("BASS_KERNEL_TRACE"))
    if trace:
        # bass_utils imports antenv.axon_hooks when trace=True; if this
        # image lacks that module, stub it (None hook -> untraced run)
        try:
            import antenv.axon_hooks  # noqa: F401
        except ImportError:
            import types

            m = types.ModuleType("antenv.axon_hooks")
            m._hook = None
            m.get_axon_ntff_profile_hook = lambda: m._hook
            m.set_axon_ntff_profile_hook = lambda h: setattr(m, "_hook", h)
            sys.modules["antenv.axon_hooks"] = m
            try:
                import antenv

                antenv.axon_hooks = m
            except ImportError:
                pass
    res = run_bass_kernel_spmd(nc, in_maps, list(range(NCORES)), trace=trace)
    LAST_RESULTS = res
    y = np.stack([res.results[k]["y"] for k in range(NCORES)])  # [K,S,TEXT,B]
    yv = y[:, :, WU:, :]                                        # [K,S,L,B]
    out = np.ascontiguousarray(
        yv.transpose(0, 1, 3, 2).reshape(T, 1)
    )
    out[0:L] = _host_prefix(inputs, L)
    return out


# revision 28
# speedup vs baseline: 1.4651x; 1.4651x over previous
"""2-layer LSTM (H=100, D=13, T=131072) + linear head on 8 TRN2 cores.

Strategy: warm-up based sequence parallelism. The sequence is split into
4096 chunks of L=32 steps; each chunk is computed independently starting
from zero state W=32 steps early (forget-gate contraction makes the
trajectory converge to the exact one well within W steps, validated at
fp32 noise floor). No cross-core communication. Each core processes 512
chunks as 2 interleaved streams of 256-wide batched recurrences
(hidden dim on partitions, chunk batch on the free axis).

All activations are sigmoids: tanh(z) = 2*sig(2z)-1 is folded into
host-side weight scaling plus one fused scalar_tensor_tensor per use.
Cell/hidden state are kept halved (c~=c/2, h~=h/2) so the fusions are
exact powers of two.
"""

import os
import sys
from contextlib import ExitStack

import numpy as np

if "/opt/trn_rl_repo" not in sys.path:
    sys.path.insert(0, "/opt/trn_rl_repo")

import concourse.bass as bass
import concourse.bacc as bacc
import concourse.tile as tile
from concourse import mybir
from concourse.bass_utils import run_bass_kernel_spmd

T, D, H = 131072, 13, 100
NCORES = 8
L, WU = 32, 16
TEXT = L + WU                      # rounds per layer
S = 2                              # streams per core
B = 256                            # chunk batch per stream
assert NCORES * S * B * L == T

F32 = mybir.dt.float32
MM_DT = mybir.dt.float32r          # 1 cyc/row on PE for N>=256
AF = mybir.ActivationFunctionType
OP = mybir.AluOpType

LAST_RESULTS = None
_NC_CACHE = {}


def _to_f32r(a):
    """Round fp32 to the fp32r grid (11-bit mantissa, RNE) host-side so the
    DMA'd bits already satisfy the FP32r matmul producer rule."""
    u = np.ascontiguousarray(a, np.float32).view(np.uint32).copy()
    low = u & np.uint32(0xFFF)
    base = u & np.uint32(0xFFFFF000)
    up = (low > 0x800) | ((low == 0x800) & (((base >> np.uint32(12)) & 1) == 1))
    base = base + np.where(up, np.uint32(0x1000), np.uint32(0))
    return base.view(np.float32)


def _build_nc():
    nc = bacc.Bacc("TRN2", target_bir_lowering=False, debug=False)
    xh_d = nc.dram_tensor(
        "xh", [S, 14, (TEXT + 1) * B], MM_DT, kind="ExternalInput"
    ).ap()
    g1_d = nc.dram_tensor("g1", [114, 400], MM_DT, kind="ExternalInput").ap()
    g2a_d = nc.dram_tensor("g2a", [114, 400], MM_DT, kind="ExternalInput").ap()
    g2b_d = nc.dram_tensor("g2b", [100, 400], MM_DT, kind="ExternalInput").ap()
    gfc_d = nc.dram_tensor("gfc", [114, 1], MM_DT, kind="ExternalInput").ap()
    y_d = nc.dram_tensor("y", [S, TEXT * B], F32, kind="ExternalOutput").ap()

    with tile.TileContext(nc) as tc, ExitStack() as ctx:
        pers = ctx.enter_context(tc.tile_pool(name="pers", bufs=1))
        # moving tile stream: rows 0:100 h~ (block t+1 written at round t),
        # rows 100:113 x_t, row 113 ones — block t is round t's matmul rhs
        h1s = [
            pers.tile([114, (TEXT + 1) * B], MM_DT, name=f"h1s{s}") for s in range(S)
        ]
        ct1 = [pers.tile([100, B], F32, name=f"ct1_{s}") for s in range(S)]
        ct2 = [pers.tile([100, B], F32, name=f"ct2_{s}") for s in range(S)]
        G1 = pers.tile([114, 400], MM_DT)
        G2a = pers.tile([114, 400], MM_DT)
        G2b = pers.tile([100, 400], MM_DT)
        Gfc = pers.tile([114, 1], MM_DT)

        psum = [
            ctx.enter_context(tc.tile_pool(name=f"ps{s}", bufs=2, space="PSUM"))
            for s in range(S)
        ]
        sigp = [
            ctx.enter_context(tc.tile_pool(name=f"sig{s}", bufs=2)) for s in range(S)
        ]
        tmpp = [
            ctx.enter_context(tc.tile_pool(name=f"tmp{s}", bufs=2)) for s in range(S)
        ]

        nc.sync.dma_start(G1[:], g1_d[:])
        nc.sync.dma_start(G2a[:], g2a_d[:])
        nc.sync.dma_start(G2b[:], g2b_d[:])
        nc.sync.dma_start(Gfc[:], gfc_d[:])
        # Memset cannot emit float32r, so the f32r h-rows of block 0 are
        # zeroed by copying from an fp32 scratch (copy converts on write).
        sc0 = pers.tile([128, B], F32, name="sc0")
        nc.vector.memset(sc0[:], 0.0)
        for s in range(S):
            nc.sync.dma_start(h1s[s][100:114, 0 : (TEXT + 1) * B], xh_d[s])
            nc.vector.tensor_copy(h1s[s][0:100, 0:B], sc0[0:100, :])
            nc.vector.memset(ct1[s][:], 0.0)
            nc.vector.memset(ct2[s][:], 0.0)

        def cell_tail(s, sig, ct, h_out):
            """it = (sig_g - 0.5)*sig_i ; ct' = sig_f*ct + it ;
            h~ = (sig(4ct') - 0.5) * sig_o  -> h_out"""
            it2 = tmpp[s].tile([100, B], F32)
            nc.vector.scalar_tensor_tensor(
                it2[:], sig[:, 2 * B : 3 * B], 0.5, sig[:, 0:B],
                op0=OP.subtract, op1=OP.mult,
            )
            fc = tmpp[s].tile([100, B], F32)
            nc.gpsimd.tensor_tensor(fc[:], sig[:, B : 2 * B], ct[:], op=OP.mult)
            nc.gpsimd.tensor_tensor(ct[:], fc[:], it2[:], op=OP.add)
            sc = tmpp[s].tile([100, B], F32)
            nc.scalar.activation(sc[:], ct[:], AF.Sigmoid, scale=4.0)
            nc.vector.scalar_tensor_tensor(
                h_out, sc[:], 0.5, sig[:, 3 * B : 4 * B],
                op0=OP.subtract, op1=OP.mult,
            )

        # ---- layer 1 ----
        for t in range(TEXT):
            for s in range(S):
                mov = h1s[s][:, t * B : (t + 1) * B]
                ps = psum[s].tile([100, 4 * B], F32)
                for g in range(4):
                    nc.tensor.matmul(
                        ps[:, g * B : (g + 1) * B],
                        G1[:, g * 100 : (g + 1) * 100],
                        mov,
                        start=True, stop=True,
                    )
                sig = sigp[s].tile([100, 4 * B], F32)
                nc.scalar.activation(sig[:], ps[:], AF.Sigmoid)
                cell_tail(s, sig, ct1[s], h1s[s][0:100, (t + 1) * B : (t + 2) * B])

        # ---- layer 2 (h~2_t overwrites h1s block t+1 after G2b reads it) ----
        for t in range(TEXT):
            for s in range(S):
                prev = h1s[s][:, t * B : (t + 1) * B]
                h1blk = h1s[s][0:100, (t + 1) * B : (t + 2) * B]
                ps = psum[s].tile([100, 4 * B], F32)
                for g in range(4):
                    sl = slice(g * B, (g + 1) * B)
                    nc.tensor.matmul(
                        ps[:, sl], G2b[:, g * 100 : (g + 1) * 100], h1blk,
                        start=True, stop=False,
                    )
                    nc.tensor.matmul(
                        ps[:, sl], G2a[:, g * 100 : (g + 1) * 100], prev,
                        start=False, stop=True,
                    )
                sig = sigp[s].tile([100, 4 * B], F32)
                nc.scalar.activation(sig[:], ps[:], AF.Sigmoid)
                cell_tail(s, sig, ct2[s], h1s[s][0:100, (t + 1) * B : (t + 2) * B])

        # ---- output head tail: y_t = Gfc.T @ h1s block t+1 ----
        for s in range(S):
            for j in range(TEXT // 4):
                ps = psum[s].tile([100, 4 * B], F32)
                for q in range(4):
                    t = 4 * j + q
                    nc.tensor.matmul(
                        ps[0:1, q * B : (q + 1) * B], Gfc[:],
                        h1s[s][:, (t + 1) * B : (t + 2) * B],
                        start=True, stop=True,
                    )
                yt = tmpp[s].tile([1, 4 * B], F32, name=f"yt{s}")
                nc.vector.tensor_copy(yt[:], ps[0:1, :])
                nc.sync.dma_start(y_d[s, 4 * j * B : (4 * j + 4) * B], yt[:])

    nc.compile()
    return nc


def get_nc():
    if "nc" not in _NC_CACHE:
        _NC_CACHE["nc"] = _build_nc()
    return _NC_CACHE["nc"]


def _prep_weights(inputs):
    f = np.float32
    G1 = np.zeros((114, 400), f)
    G1[0:100] = 2.0 * inputs["W_hh1"].T
    G1[100:113] = inputs["W_ih1"].T
    G1[113] = inputs["b_ih1"] + inputs["b_hh1"]
    G1[:, 200:300] *= 2.0
    G2a = np.zeros((114, 400), f)
    G2a[0:100] = 2.0 * inputs["W_hh2"].T
    G2a[113] = inputs["b_ih2"] + inputs["b_hh2"]
    G2a[:, 200:300] *= 2.0
    G2b = np.ascontiguousarray(2.0 * inputs["W_ih2"].T).astype(f)
    G2b[:, 200:300] *= 2.0
    Gfc = np.zeros((114, 1), f)
    Gfc[0:100, 0] = 2.0 * inputs["W_fc"][0]
    Gfc[113, 0] = inputs["b_fc"][0]
    return G1, G2a, G2b, Gfc


def _prep_x(x):
    f = np.float32
    c_all = np.arange(NCORES * S * B).reshape(NCORES, S, B)
    gidx = c_all[..., None] * L - WU + np.arange(TEXT)          # [K,S,B,TEXT]
    valid = gidx >= 0
    xv = np.where(
        valid[..., None], x[np.clip(gidx, 0, T - 1)], np.float32(0.0)
    )                                                           # [K,S,B,TEXT,13]
    xh = np.empty((NCORES, S, 14, TEXT + 1, B), f)
    xh[:, :, 0:13, :TEXT] = xv.transpose(0, 1, 4, 3, 2)
    xh[:, :, 0:13, TEXT] = 0.0
    xh[:, :, 13] = 1.0
    return xh.reshape(NCORES, S, 14, (TEXT + 1) * B)


def _host_prefix(inputs, n):
    """Exact first-n outputs in plain numpy (chunk 0 has no warm-up window)."""
    f = np.float32
    x = inputs["x"].astype(f)

    def sig(v):
        return 1.0 / (1.0 + np.exp(-v))

    def layer(xs, Wih, Whh, b):
        h = np.zeros(H, f)
        c = np.zeros(H, f)
        hs = np.empty((n, H), f)
        for t in range(n):
            g = (Wih @ xs[t] + b + Whh @ h).astype(f)
            i, fg, gg, o = g[:H], g[H : 2 * H], g[2 * H : 3 * H], g[3 * H :]
            c = (sig(fg) * c + sig(i) * np.tanh(gg)).astype(f)
            h = (sig(o) * np.tanh(c)).astype(f)
            hs[t] = h
        return hs

    h1 = layer(x[:n], inputs["W_ih1"], inputs["W_hh1"], inputs["b_ih1"] + inputs["b_hh1"])
    h2 = layer(h1, inputs["W_ih2"], inputs["W_hh2"], inputs["b_ih2"] + inputs["b_hh2"])
    return (h2 @ inputs["W_fc"].T + inputs["b_fc"]).astype(f)


def kernel(**inputs) -> np.ndarray:
    global LAST_RESULTS
    inputs = {k: np.asarray(v, dtype=np.float32) for k, v in inputs.items()}
    G1, G2a, G2b, Gfc = (_to_f32r(g) for g in _prep_weights(inputs))
    xh = _to_f32r(_prep_x(inputs["x"]))
    in_maps = [
        {"xh": xh[k], "g1": G1, "g2a": G2a, "g2b": G2b, "gfc": Gfc}
        for k in range(NCORES)
    ]
    nc = get_nc()
    trace = bool(os.environ.get("BASS_KERNEL_TRACE"))
    if trace:
        # bass_utils imports antenv.axon_hooks when trace=True; if this
        # image lacks that module, stub it (None hook -> untraced run)
        try:
            import antenv.axon_hooks  # noqa: F401
        except ImportError:
            import types

            m = types.ModuleType("antenv.axon_hooks")
            m._hook = None
            m.get_axon_ntff_profile_hook = lambda: m._hook
            m.set_axon_ntff_profile_hook = lambda h: setattr(m, "_hook", h)
            sys.modules["antenv.axon_hooks"] = m
            try:
                import antenv

                antenv.axon_hooks = m
            except ImportError:
                pass
    res = run_bass_kernel_spmd(nc, in_maps, list(range(NCORES)), trace=trace)
    LAST_RESULTS = res
    y = np.stack([res.results[k]["y"] for k in range(NCORES)]).reshape(
        NCORES, S, TEXT, B
    )
    yv = y[:, :, WU:, :]                                        # [K,S,L,B]
    out = np.ascontiguousarray(
        yv.transpose(0, 1, 3, 2).reshape(T, 1)
    )
    out[0:L] = _host_prefix(inputs, L)
    return out


# revision 30
# speedup vs baseline: 2.2050x; 1.5050x over previous
"""2-layer LSTM (H=100, D=13, T=131072) + linear head on 8 TRN2 cores.

Strategy: warm-up based sequence parallelism. The sequence is split into
4096 chunks of L=32 steps; each chunk is computed independently starting
from zero state W=32 steps early (forget-gate contraction makes the
trajectory converge to the exact one well within W steps, validated at
fp32 noise floor). No cross-core communication. Each core processes 512
chunks as 2 interleaved streams of 256-wide batched recurrences
(hidden dim on partitions, chunk batch on the free axis).

All activations are sigmoids: tanh(z) = 2*sig(2z)-1 is folded into
host-side weight scaling plus one fused scalar_tensor_tensor per use.
Cell/hidden state are kept halved (c~=c/2, h~=h/2) so the fusions are
exact powers of two.
"""

import os
import sys
from contextlib import ExitStack

import numpy as np

if "/opt/trn_rl_repo" not in sys.path:
    sys.path.insert(0, "/opt/trn_rl_repo")

import concourse.bass as bass
import concourse.bacc as bacc
import concourse.tile as tile
from concourse import mybir
from concourse.bass_utils import run_bass_kernel_spmd

T, D, H = 131072, 13, 100
NCORES = 8
L, WU = 32, 16
TEXT = L + WU                      # rounds per layer
S = 2                              # streams per core
B = 256                            # chunk batch per stream
assert NCORES * S * B * L == T

F32 = mybir.dt.float32
MM_DT = mybir.dt.float32r          # 1 cyc/row on PE for N>=256
AF = mybir.ActivationFunctionType
OP = mybir.AluOpType

LAST_RESULTS = None
_NC_CACHE = {}


def _to_f32r(a):
    """Round fp32 to the fp32r grid (11-bit mantissa, RNE) host-side so the
    DMA'd bits already satisfy the FP32r matmul producer rule."""
    u = np.ascontiguousarray(a, np.float32).view(np.uint32).copy()
    low = u & np.uint32(0xFFF)
    base = u & np.uint32(0xFFFFF000)
    up = (low > 0x800) | ((low == 0x800) & (((base >> np.uint32(12)) & 1) == 1))
    base = base + np.where(up, np.uint32(0x1000), np.uint32(0))
    return base.view(np.float32)


def _build_nc():
    nc = bacc.Bacc("TRN2", target_bir_lowering=False, debug=False)
    xh_d = nc.dram_tensor(
        "xh", [S, 14, (TEXT + 1) * B], MM_DT, kind="ExternalInput"
    ).ap()
    g1_d = nc.dram_tensor("g1", [114, 400], MM_DT, kind="ExternalInput").ap()
    g2a_d = nc.dram_tensor("g2a", [114, 400], MM_DT, kind="ExternalInput").ap()
    g2b_d = nc.dram_tensor("g2b", [100, 400], MM_DT, kind="ExternalInput").ap()
    gfc_d = nc.dram_tensor("gfc", [114, 1], MM_DT, kind="ExternalInput").ap()
    y_d = nc.dram_tensor("y", [S, TEXT * B], F32, kind="ExternalOutput").ap()

    with tile.TileContext(nc) as tc, ExitStack() as ctx:
        pers = ctx.enter_context(tc.tile_pool(name="pers", bufs=1))
        # moving tile stream: rows 0:100 h~ (block t+1 written at round t),
        # rows 100:113 x_t, row 113 ones — block t is round t's matmul rhs
        h1s = [
            pers.tile([114, (TEXT + 1) * B], MM_DT, name=f"h1s{s}") for s in range(S)
        ]
        ct1 = [pers.tile([100, B], F32, name=f"ct1_{s}") for s in range(S)]
        ct2 = [pers.tile([100, B], F32, name=f"ct2_{s}") for s in range(S)]
        G1 = pers.tile([114, 400], MM_DT)
        G2a = pers.tile([114, 400], MM_DT)
        G2b = pers.tile([100, 400], MM_DT)
        Gfc = pers.tile([114, 1], MM_DT)

        psum = [
            ctx.enter_context(tc.tile_pool(name=f"ps{s}", bufs=2, space="PSUM"))
            for s in range(S)
        ]
        sigp = [
            ctx.enter_context(tc.tile_pool(name=f"sig{s}", bufs=3)) for s in range(S)
        ]
        tmpp = [
            ctx.enter_context(tc.tile_pool(name=f"tmp{s}", bufs=4)) for s in range(S)
        ]

        nc.sync.dma_start(G1[:], g1_d[:])
        nc.sync.dma_start(G2a[:], g2a_d[:])
        nc.sync.dma_start(G2b[:], g2b_d[:])
        nc.sync.dma_start(Gfc[:], gfc_d[:])
        # Memset cannot emit float32r, so the f32r h-rows of block 0 are
        # zeroed by copying from an fp32 scratch (copy converts on write).
        sc0 = pers.tile([128, B], F32, name="sc0")
        nc.vector.memset(sc0[:], 0.0)
        for s in range(S):
            nc.sync.dma_start(h1s[s][100:114, 0 : (TEXT + 1) * B], xh_d[s])
            nc.vector.tensor_copy(h1s[s][0:100, 0:B], sc0[0:100, :])
            nc.vector.memset(ct1[s][:], 0.0)
            nc.vector.memset(ct2[s][:], 0.0)

        def cell_tail(s, sig, ct, h_out):
            """it = (sig_g - 0.5)*sig_i ; ct' = sig_f*ct + it ;
            h~ = (sig(4ct') - 0.5) * sig_o  -> h_out"""
            it2 = tmpp[s].tile([100, B], F32)
            nc.vector.scalar_tensor_tensor(
                it2[:], sig[:, 2 * B : 3 * B], 0.5, sig[:, 0:B],
                op0=OP.subtract, op1=OP.mult,
            )
            fc = tmpp[s].tile([100, B], F32)
            nc.gpsimd.tensor_tensor(fc[:], sig[:, B : 2 * B], ct[:], op=OP.mult)
            nc.gpsimd.tensor_tensor(ct[:], fc[:], it2[:], op=OP.add)
            sc = tmpp[s].tile([100, B], F32)
            nc.scalar.activation(sc[:], ct[:], AF.Sigmoid, scale=4.0)
            nc.vector.scalar_tensor_tensor(
                h_out, sc[:], 0.5, sig[:, 3 * B : 4 * B],
                op0=OP.subtract, op1=OP.mult,
            )

        # ---- layers 1+2 interleaved: L2 lags L1 by one round, giving 4
        # independent recurrence chains (2 streams x 2 layers) in flight.
        # L2 round u overwrites h1s block u+1 h-rows (h~1_u -> h~2_u); the
        # WAR on L1 round u+1's read of block u+1 is tracked by Tile and
        # satisfied naturally since L1's matmul comes first in program order.
        def l1_round(t, s):
            mov = h1s[s][:, t * B : (t + 1) * B]
            ps = psum[s].tile([100, 4 * B], F32)
            for g in range(4):
                nc.tensor.matmul(
                    ps[:, g * B : (g + 1) * B],
                    G1[:, g * 100 : (g + 1) * 100],
                    mov,
                    start=True, stop=True,
                )
            sig = sigp[s].tile([100, 4 * B], F32)
            nc.scalar.activation(sig[:], ps[:], AF.Sigmoid)
            cell_tail(s, sig, ct1[s], h1s[s][0:100, (t + 1) * B : (t + 2) * B])

        def l2_round(u, s):
            prev = h1s[s][:, u * B : (u + 1) * B]
            h1blk = h1s[s][0:100, (u + 1) * B : (u + 2) * B]
            ps = psum[s].tile([100, 4 * B], F32)
            for g in range(4):
                sl = slice(g * B, (g + 1) * B)
                nc.tensor.matmul(
                    ps[:, sl], G2b[:, g * 100 : (g + 1) * 100], h1blk,
                    start=True, stop=False,
                )
                nc.tensor.matmul(
                    ps[:, sl], G2a[:, g * 100 : (g + 1) * 100], prev,
                    start=False, stop=True,
                )
            sig = sigp[s].tile([100, 4 * B], F32)
            nc.scalar.activation(sig[:], ps[:], AF.Sigmoid)
            cell_tail(s, sig, ct2[s], h1s[s][0:100, (u + 1) * B : (u + 2) * B])

        for t in range(TEXT + 1):
            for s in range(S):
                if t < TEXT:
                    l1_round(t, s)
                if t >= 1:
                    l2_round(t - 1, s)

        # ---- output head tail: y_t = Gfc.T @ h1s block t+1 ----
        for s in range(S):
            for j in range(TEXT // 4):
                ps = psum[s].tile([100, 4 * B], F32)
                for q in range(4):
                    t = 4 * j + q
                    nc.tensor.matmul(
                        ps[0:1, q * B : (q + 1) * B], Gfc[:],
                        h1s[s][:, (t + 1) * B : (t + 2) * B],
                        start=True, stop=True,
                    )
                yt = tmpp[s].tile([1, 4 * B], F32, name=f"yt{s}")
                nc.vector.tensor_copy(yt[:], ps[0:1, :])
                nc.sync.dma_start(y_d[s, 4 * j * B : (4 * j + 4) * B], yt[:])

    nc.compile()
    return nc


def get_nc():
    if "nc" not in _NC_CACHE:
        _NC_CACHE["nc"] = _build_nc()
    return _NC_CACHE["nc"]


def _prep_weights(inputs):
    f = np.float32
    G1 = np.zeros((114, 400), f)
    G1[0:100] = 2.0 * inputs["W_hh1"].T
    G1[100:113] = inputs["W_ih1"].T
    G1[113] = inputs["b_ih1"] + inputs["b_hh1"]
    G1[:, 200:300] *= 2.0
    G2a = np.zeros((114, 400), f)
    G2a[0:100] = 2.0 * inputs["W_hh2"].T
    G2a[113] = inputs["b_ih2"] + inputs["b_hh2"]
    G2a[:, 200:300] *= 2.0
    G2b = np.ascontiguousarray(2.0 * inputs["W_ih2"].T).astype(f)
    G2b[:, 200:300] *= 2.0
    Gfc = np.zeros((114, 1), f)
    Gfc[0:100, 0] = 2.0 * inputs["W_fc"][0]
    Gfc[113, 0] = inputs["b_fc"][0]
    return G1, G2a, G2b, Gfc


def _prep_x(x):
    f = np.float32
    c_all = np.arange(NCORES * S * B).reshape(NCORES, S, B)
    gidx = c_all[..., None] * L - WU + np.arange(TEXT)          # [K,S,B,TEXT]
    valid = gidx >= 0
    xv = np.where(
        valid[..., None], x[np.clip(gidx, 0, T - 1)], np.float32(0.0)
    )                                                           # [K,S,B,TEXT,13]
    xh = np.empty((NCORES, S, 14, TEXT + 1, B), f)
    xh[:, :, 0:13, :TEXT] = xv.transpose(0, 1, 4, 3, 2)
    xh[:, :, 0:13, TEXT] = 0.0
    xh[:, :, 13] = 1.0
    return xh.reshape(NCORES, S, 14, (TEXT + 1) * B)


def _host_prefix(inputs, n):
    """Exact first-n outputs in plain numpy (chunk 0 has no warm-up window)."""
    f = np.float32
    x = inputs["x"].astype(f)

    def sig(v):
        return 1.0 / (1.0 + np.exp(-v))

    def layer(xs, Wih, Whh, b):
        h = np.zeros(H, f)
        c = np.zeros(H, f)
        hs = np.empty((n, H), f)
        for t in range(n):
            g = (Wih @ xs[t] + b + Whh @ h).astype(f)
            i, fg, gg, o = g[:H], g[H : 2 * H], g[2 * H : 3 * H], g[3 * H :]
            c = (sig(fg) * c + sig(i) * np.tanh(gg)).astype(f)
            h = (sig(o) * np.tanh(c)).astype(f)
            hs[t] = h
        return hs

    h1 = layer(x[:n], inputs["W_ih1"], inputs["W_hh1"], inputs["b_ih1"] + inputs["b_hh1"])
    h2 = layer(h1, inputs["W_ih2"], inputs["W_hh2"], inputs["b_ih2"] + inputs["b_hh2"])
    return (h2 @ inputs["W_fc"].T + inputs["b_fc"]).astype(f)


def kernel(**inputs) -> np.ndarray:
    global LAST_RESULTS
    inputs = {k: np.asarray(v, dtype=np.float32) for k, v in inputs.items()}
    G1, G2a, G2b, Gfc = (_to_f32r(g) for g in _prep_weights(inputs))
    xh = _to_f32r(_prep_x(inputs["x"]))
    in_maps = [
        {"xh": xh[k], "g1": G1, "g2a": G2a, "g2b": G2b, "gfc": Gfc}
        for k in range(NCORES)
    ]
    nc = get_nc()
    trace = bool(os.environ.get("BASS_KERNEL_TRACE"))
    if trace:
        # bass_utils imports antenv.axon_hooks when trace=True; if this
        # image lacks that module, stub it (None hook -> untraced run)
        try:
            import antenv.axon_hooks  # noqa: F401
        except ImportError:
            import types

            m = types.ModuleType("antenv.axon_hooks")
            m._hook = None
            m.get_axon_ntff_profile_hook = lambda: m._hook
            m.set_axon_ntff_profile_hook = lambda h: setattr(m, "_hook", h)
            sys.modules["antenv.axon_hooks"] = m
            try:
                import antenv

                antenv.axon_hooks = m
            except ImportError:
                pass
    res = run_bass_kernel_spmd(nc, in_maps, list(range(NCORES)), trace=trace)
    LAST_RESULTS = res
    y = np.stack([res.results[k]["y"] for k in range(NCORES)]).reshape(
        NCORES, S, TEXT, B
    )
    yv = y[:, :, WU:, :]                                        # [K,S,L,B]
    out = np.ascontiguousarray(
        yv.transpose(0, 1, 3, 2).reshape(T, 1)
    )
    out[0:L] = _host_prefix(inputs, L)
    return out


# revision 34
# speedup vs baseline: 2.2789x; 1.0335x over previous
"""2-layer LSTM (H=100, D=13, T=131072) + linear head on 8 TRN2 cores.

Strategy: warm-up based sequence parallelism. The sequence is split into
4096 chunks of L=32 steps; each chunk is computed independently starting
from zero state W=32 steps early (forget-gate contraction makes the
trajectory converge to the exact one well within W steps, validated at
fp32 noise floor). No cross-core communication. Each core processes 512
chunks as 2 interleaved streams of 256-wide batched recurrences
(hidden dim on partitions, chunk batch on the free axis).

All activations are sigmoids: tanh(z) = 2*sig(2z)-1 is folded into
host-side weight scaling plus one fused scalar_tensor_tensor per use.
Cell/hidden state are kept halved (c~=c/2, h~=h/2) so the fusions are
exact powers of two.
"""

import os
import sys
from contextlib import ExitStack

import numpy as np

if "/opt/trn_rl_repo" not in sys.path:
    sys.path.insert(0, "/opt/trn_rl_repo")

import concourse.bass as bass
import concourse.bacc as bacc
import concourse.tile as tile
from concourse import mybir
from concourse.bass_utils import run_bass_kernel_spmd

T, D, H = 131072, 13, 100
NCORES = 8
L, WU = 32, 16
TEXT = L + WU                      # rounds per layer
S = 2                              # streams per core
B = 256                            # chunk batch per stream
assert NCORES * S * B * L == T

F32 = mybir.dt.float32
MM_DT = mybir.dt.float32r          # 1 cyc/row on PE for N>=256
AF = mybir.ActivationFunctionType
OP = mybir.AluOpType

LAST_RESULTS = None
_NC_CACHE = {}


def _to_f32r(a):
    """Round fp32 to the fp32r grid (11-bit mantissa, RNE) host-side so the
    DMA'd bits already satisfy the FP32r matmul producer rule."""
    u = np.ascontiguousarray(a, np.float32).view(np.uint32).copy()
    low = u & np.uint32(0xFFF)
    base = u & np.uint32(0xFFFFF000)
    up = (low > 0x800) | ((low == 0x800) & (((base >> np.uint32(12)) & 1) == 1))
    base = base + np.where(up, np.uint32(0x1000), np.uint32(0))
    return base.view(np.float32)


def _build_nc():
    nc = bacc.Bacc("TRN2", target_bir_lowering=False, debug=False)
    xh_d = nc.dram_tensor(
        "xh", [S, 14, (TEXT + 1) * B], MM_DT, kind="ExternalInput"
    ).ap()
    g1_d = nc.dram_tensor("g1", [114, 400], MM_DT, kind="ExternalInput").ap()
    g2a_d = nc.dram_tensor("g2a", [114, 400], MM_DT, kind="ExternalInput").ap()
    g2b_d = nc.dram_tensor("g2b", [100, 400], MM_DT, kind="ExternalInput").ap()
    gfc_d = nc.dram_tensor("gfc", [114, 1], MM_DT, kind="ExternalInput").ap()
    y_d = nc.dram_tensor("y", [S, TEXT * B], F32, kind="ExternalOutput").ap()

    with tile.TileContext(nc) as tc, ExitStack() as ctx:
        pers = ctx.enter_context(tc.tile_pool(name="pers", bufs=1))
        # moving tile stream: rows 0:100 h~ (block t+1 written at round t),
        # rows 100:113 x_t, row 113 ones — block t is round t's matmul rhs
        h1s = [
            pers.tile([114, (TEXT + 1) * B], MM_DT, name=f"h1s{s}") for s in range(S)
        ]
        ct1 = [pers.tile([100, B], F32, name=f"ct1_{s}") for s in range(S)]
        ct2 = [pers.tile([100, B], F32, name=f"ct2_{s}") for s in range(S)]
        G1 = pers.tile([114, 400], MM_DT)
        G2a = pers.tile([114, 400], MM_DT)
        G2b = pers.tile([100, 400], MM_DT)
        Gfc = pers.tile([114, 1], MM_DT)

        psum = [
            ctx.enter_context(tc.tile_pool(name=f"ps{s}", bufs=2, space="PSUM"))
            for s in range(S)
        ]
        sigp = [
            ctx.enter_context(tc.tile_pool(name=f"sig{s}", bufs=3)) for s in range(S)
        ]
        tmpp = [
            ctx.enter_context(tc.tile_pool(name=f"tmp{s}", bufs=4)) for s in range(S)
        ]

        nc.sync.dma_start(G1[:], g1_d[:])
        nc.sync.dma_start(G2a[:], g2a_d[:])
        nc.sync.dma_start(G2b[:], g2b_d[:])
        nc.sync.dma_start(Gfc[:], gfc_d[:])
        # Memset cannot emit float32r, so the f32r h-rows of block 0 are
        # zeroed by copying from an fp32 scratch (copy converts on write).
        sc0 = pers.tile([128, B], F32, name="sc0")
        nc.vector.memset(sc0[:], 0.0)
        for s in range(S):
            nc.sync.dma_start(h1s[s][100:114, 0 : (TEXT + 1) * B], xh_d[s])
            nc.vector.tensor_copy(h1s[s][0:100, 0:B], sc0[0:100, :])
            nc.vector.memset(ct1[s][:], 0.0)
            nc.vector.memset(ct2[s][:], 0.0)

        def cell_tail(s, sig, ct, h_out, dve_ct=False):
            """it = (sig_g - 0.5)*sig_i ; ct' = sig_f*ct + it ;
            h~ = (sig(4ct') - 0.5) * sig_o  -> h_out"""
            it2 = tmpp[s].tile([100, B], F32)
            nc.vector.scalar_tensor_tensor(
                it2[:], sig[:, 2 * B : 3 * B], 0.5, sig[:, 0:B],
                op0=OP.subtract, op1=OP.mult,
            )
            fc = tmpp[s].tile([100, B], F32)
            nc.gpsimd.tensor_tensor(fc[:], sig[:, B : 2 * B], ct[:], op=OP.mult)
            eng = nc.vector if dve_ct else nc.gpsimd
            eng.tensor_tensor(ct[:], fc[:], it2[:], op=OP.add)
            sc = tmpp[s].tile([100, B], F32)
            nc.scalar.activation(sc[:], ct[:], AF.Sigmoid, scale=4.0)
            nc.vector.scalar_tensor_tensor(
                h_out, sc[:], 0.5, sig[:, 3 * B : 4 * B],
                op0=OP.subtract, op1=OP.mult,
            )

        # ---- layers 1+2 interleaved: L2 lags L1 by one round, giving 4
        # independent recurrence chains (2 streams x 2 layers) in flight.
        # L2 round u overwrites h1s block u+1 h-rows (h~1_u -> h~2_u); the
        # WAR on L1 round u+1's read of block u+1 is tracked by Tile and
        # satisfied naturally since L1's matmul comes first in program order.
        def l1_round(t, s):
            mov = h1s[s][:, t * B : (t + 1) * B]
            ps = psum[s].tile([100, 4 * B], F32)
            for g in range(4):
                nc.tensor.matmul(
                    ps[:, g * B : (g + 1) * B],
                    G1[:, g * 100 : (g + 1) * 100],
                    mov,
                    start=True, stop=True,
                )
            sig = sigp[s].tile([100, 4 * B], F32)
            nc.scalar.activation(sig[:], ps[:], AF.Sigmoid)
            cell_tail(s, sig, ct1[s], h1s[s][0:100, (t + 1) * B : (t + 2) * B])

        def l2_round(u, s):
            prev = h1s[s][:, u * B : (u + 1) * B]
            h1blk = h1s[s][0:100, (u + 1) * B : (u + 2) * B]
            ps = psum[s].tile([100, 4 * B], F32)
            for g in range(4):
                sl = slice(g * B, (g + 1) * B)
                nc.tensor.matmul(
                    ps[:, sl], G2b[:, g * 100 : (g + 1) * 100], h1blk,
                    start=True, stop=False,
                )
                nc.tensor.matmul(
                    ps[:, sl], G2a[:, g * 100 : (g + 1) * 100], prev,
                    start=False, stop=True,
                )
            sig = sigp[s].tile([100, 4 * B], F32)
            nc.scalar.activation(sig[:], ps[:], AF.Sigmoid)
            cell_tail(
                s, sig, ct2[s], h1s[s][0:100, (u + 1) * B : (u + 2) * B], dve_ct=True
            )

        for t in range(TEXT + 1):
            for s in range(S):
                if t < TEXT:
                    l1_round(t, s)
                if t >= 1:
                    l2_round(t - 1, s)

        # ---- output head tail: y_t = Gfc.T @ h1s block t+1 ----
        for s in range(S):
            for j in range(TEXT // 4):
                ps = psum[s].tile([100, 4 * B], F32)
                for hlf in range(2):
                    t = 4 * j + 2 * hlf
                    nc.tensor.matmul(
                        ps[0:1, hlf * 2 * B : (hlf + 1) * 2 * B], Gfc[:],
                        h1s[s][:, (t + 1) * B : (t + 3) * B],
                        start=True, stop=True,
                    )
                yt = tmpp[s].tile([1, 4 * B], F32, name=f"yt{s}")
                nc.vector.tensor_copy(yt[:], ps[0:1, :])
                nc.sync.dma_start(y_d[s, 4 * j * B : (4 * j + 4) * B], yt[:])

    nc.compile()
    return nc


def get_nc():
    if "nc" not in _NC_CACHE:
        _NC_CACHE["nc"] = _build_nc()
    return _NC_CACHE["nc"]


def _prep_weights(inputs):
    f = np.float32
    G1 = np.zeros((114, 400), f)
    G1[0:100] = 2.0 * inputs["W_hh1"].T
    G1[100:113] = inputs["W_ih1"].T
    G1[113] = inputs["b_ih1"] + inputs["b_hh1"]
    G1[:, 200:300] *= 2.0
    G2a = np.zeros((114, 400), f)
    G2a[0:100] = 2.0 * inputs["W_hh2"].T
    G2a[113] = inputs["b_ih2"] + inputs["b_hh2"]
    G2a[:, 200:300] *= 2.0
    G2b = np.ascontiguousarray(2.0 * inputs["W_ih2"].T).astype(f)
    G2b[:, 200:300] *= 2.0
    Gfc = np.zeros((114, 1), f)
    Gfc[0:100, 0] = 2.0 * inputs["W_fc"][0]
    Gfc[113, 0] = inputs["b_fc"][0]
    return G1, G2a, G2b, Gfc


def _prep_x(x):
    f = np.float32
    c_all = np.arange(NCORES * S * B).reshape(NCORES, S, B)
    gidx = c_all[..., None] * L - WU + np.arange(TEXT)          # [K,S,B,TEXT]
    valid = gidx >= 0
    xv = np.where(
        valid[..., None], x[np.clip(gidx, 0, T - 1)], np.float32(0.0)
    )                                                           # [K,S,B,TEXT,13]
    xh = np.empty((NCORES, S, 14, TEXT + 1, B), f)
    xh[:, :, 0:13, :TEXT] = xv.transpose(0, 1, 4, 3, 2)
    xh[:, :, 0:13, TEXT] = 0.0
    xh[:, :, 13] = 1.0
    return xh.reshape(NCORES, S, 14, (TEXT + 1) * B)


def _host_prefix(inputs, n):
    """Exact first-n outputs in plain numpy (chunk 0 has no warm-up window)."""
    f = np.float32
    x = inputs["x"].astype(f)

    def sig(v):
        return 1.0 / (1.0 + np.exp(-v))

    def layer(xs, Wih, Whh, b):
        h = np.zeros(H, f)
        c = np.zeros(H, f)
        hs = np.empty((n, H), f)
        for t in range(n):
            g = (Wih @ xs[t] + b + Whh @ h).astype(f)
            i, fg, gg, o = g[:H], g[H : 2 * H], g[2 * H : 3 * H], g[3 * H :]
            c = (sig(fg) * c + sig(i) * np.tanh(gg)).astype(f)
            h = (sig(o) * np.tanh(c)).astype(f)
            hs[t] = h
        return hs

    h1 = layer(x[:n], inputs["W_ih1"], inputs["W_hh1"], inputs["b_ih1"] + inputs["b_hh1"])
    h2 = layer(h1, inputs["W_ih2"], inputs["W_hh2"], inputs["b_ih2"] + inputs["b_hh2"])
    return (h2 @ inputs["W_fc"].T + inputs["b_fc"]).astype(f)


def kernel(**inputs) -> np.ndarray:
    global LAST_RESULTS
    inputs = {k: np.asarray(v, dtype=np.float32) for k, v in inputs.items()}
    G1, G2a, G2b, Gfc = (_to_f32r(g) for g in _prep_weights(inputs))
    xh = _to_f32r(_prep_x(inputs["x"]))
    in_maps = [
        {"xh": xh[k], "g1": G1, "g2a": G2a, "g2b": G2b, "gfc": Gfc}
        for k in range(NCORES)
    ]
    nc = get_nc()
    trace = bool(os.environ.get("BASS_KERNEL_TRACE"))
    if trace:
        # bass_utils imports antenv.axon_hooks when trace=True; if this
        # image lacks that module, stub it (None hook -> untraced run)
        try:
            import antenv.axon_hooks  # noqa: F401
        except ImportError:
            import types

            m = types.ModuleType("antenv.axon_hooks")
            m._hook = None
            m.get_axon_ntff_profile_hook = lambda: m._hook
            m.set_axon_ntff_profile_hook = lambda h: setattr(m, "_hook", h)
            sys.modules["antenv.axon_hooks"] = m
            try:
                import antenv

                antenv.axon_hooks = m
            except ImportError:
                pass
    res = run_bass_kernel_spmd(nc, in_maps, list(range(NCORES)), trace=trace)
    LAST_RESULTS = res
    y = np.stack([res.results[k]["y"] for k in range(NCORES)]).reshape(
        NCORES, S, TEXT, B
    )
    yv = y[:, :, WU:, :]                                        # [K,S,L,B]
    out = np.ascontiguousarray(
        yv.transpose(0, 1, 3, 2).reshape(T, 1)
    )
    out[0:L] = _host_prefix(inputs, L)
    return out
